# revision 1
# baseline (speedup 1.0000x reference)
"""Trainium2 Bass kernel for nn_CCMetrics (connected-component soft-Dice).

Math
----
Reference per sample: probs = softmax(y_pred, ch axis 1) with C=2 channels,
one-hot labels y in {0,1}.  Per-voxel channel sums collapse:
  psum_v = tsum_v = 1          (softmax / one-hot sum to 1 over channels)
  inter_v = probs[true_ch] = sigmoid((2y-1) * (z1 - z0))
So per segment id k (voronoi component, 0..64):
  inter_k = sum of sigmoid values over voxels with id k
  cnt_k   = voxel count with id k
  dice_k  = (2*inter_k + eps) / (2*cnt_k + eps)
  score   = mean over present k in 1..64;  output = mean over batch.

Device algorithm (per core, data-parallel over 4M voxels / 8 cores)
------------------------------------------------------------------
Build two packed streams per voxel (id g, value v = sigmoid(...)):
  z  = g + 0.5                  (exact half-integers)
  x' = g + 0.5 + v              (value stream, thresholds at k+0.5)
Cumulative families, one instruction per bin k (per-partition accumulate):
  R_k = sum relu(x' - (k+0.5))           [ACT Relu + bias + accum]
  T_k = #{x' >= k+0.5} = #{g >= k}       [DVE tensor_scalar is_ge + accum]
  F_k = sum sigmoid(30*(z-(k+0.5)))      [ACT Sigmoid + bias + accum]
        = 0.5*cnt_k + T_{k+1}   (exact to ~1e-13: args are multiples of 30)
Recovery (host, float64):  M_k = R_k - R_{k+1} = inter_k + T_{k+1};
walking k = 64..1 with T_65 = 0: exact T anchors from DVE bins, F bins give
cnt_k = 2*(F_k - T_{k+1}).  ACT pipelines accumulate passes at ~1.3 us while
DVE accumulate passes have a ~4.3 us drain period, so ACT takes the relu
family plus most count bins (sigmoid) and DVE takes preprocessing plus a
spread subset of exact-count anchor bins.
"""

import os
import sys

import numpy as np

for _p in ("/opt/trn_rl_repo",):
    if os.path.isdir(_p) and _p not in sys.path:
        sys.path.insert(0, _p)

from concourse import bacc, bass, mybir, tile  # noqa: E402
from concourse import bass_utils  # noqa: E402

NUM_COMP = 64
EPS = 1e-5
B, C, H, W, D = 2, 2, 128, 128, 128
N = H * W * D
NCORES = 8
CORES_PER_SAMPLE = NCORES // B
CHUNK = N // CORES_PER_SAMPLE
P = 128
F = CHUNK // P
KMAX = NUM_COMP

# Exact-count anchor bins computed on DVE (tensor_scalar is_ge + accum).
# Spread so that sigmoid-chain reconstruction segments stay short.
_nd = int(os.environ.get("CC_ND", "62"))
if _nd >= KMAX:
    DVE_BINS = frozenset(range(1, KMAX + 1))
else:
    # evenly spread anchors from k=KMAX downward
    _step = max(1, round(KMAX / max(_nd, 1)))
    DVE_BINS = frozenset(
        k for k in range(KMAX, 0, -_step)
    ) | {KMAX}
    DVE_BINS = frozenset(sorted(DVE_BINS, reverse=True)[:max(_nd, 1)])
TRACE = False

_prog_cache = {}


def _build_program():
    nc = bacc.Bacc(
        "TRN2",
        target_bir_lowering=False,
        debug=False,
        enable_asserts=False,
        num_devices=NCORES,
    )
    f32 = mybir.dt.float32
    u8 = mybir.dt.uint8

    z0_d = nc.dram_tensor("z0", [P, F], f32, kind="ExternalInput").ap()
    z1_d = nc.dram_tensor("z1", [P, F], f32, kind="ExternalInput").ap()
    y_d = nc.dram_tensor("yb", [P, F], u8, kind="ExternalInput").ap()
    g_d = nc.dram_tensor("vor", [P, F], u8, kind="ExternalInput").ap()
    # bias constants: col j (j=0..63): -(j+1.5) for relu; col 64: 0.0
    negk_d = nc.dram_tensor("negk", [P, KMAX + 1], f32, kind="ExternalInput").ap()
    # sigmoid bias constants: col j: -30*(j+1.5)
    sigb_d = nc.dram_tensor("sigb", [P, KMAX], f32, kind="ExternalInput").ap()
    out_d = nc.dram_tensor("out", [P, 3 * KMAX], f32, kind="ExternalOutput").ap()

    Alu = mybir.AluOpType
    Act = mybir.ActivationFunctionType

    with tile.TileContext(nc) as tc:
        with tc.tile_pool(name="main", bufs=1) as pool:
            z0 = pool.tile([P, F], f32)
            z1 = pool.tile([P, F], f32)
            yt = pool.tile([P, F], u8)
            gt = pool.tile([P, F], u8)
            negk = pool.tile([P, KMAX + 1], f32)
            sigb = pool.tile([P, KMAX], f32)
            # small index/const tensors first: the early ACT sigmoid block
            # needs only gt/sigb, so it must not wait behind the 4MB z DMAs
            nc.sync.dma_start(out=gt[:], in_=g_d[:])
            nc.sync.dma_start(out=sigb[:], in_=sigb_d[:])
            nc.sync.dma_start(out=negk[:], in_=negk_d[:])
            nc.sync.dma_start(out=yt[:], in_=y_d[:])
            nc.sync.dma_start(out=z0[:], in_=z0_d[:])
            nc.sync.dma_start(out=z1[:], in_=z1_d[:])

            # ---- preprocessing (DVE) ----
            zt = pool.tile([P, F], f32, tag="ru5")
            nc.vector.tensor_scalar(
                out=zt[:], in0=gt[:], scalar1=0.5, scalar2=None, op0=Alu.add,
            )
            s = pool.tile([P, F], f32, tag="ru1")
            nc.vector.tensor_sub(s[:], z1[:], z0[:])
            yf = pool.tile([P, F], f32, tag="ru2")
            nc.vector.tensor_scalar(
                out=yf[:], in0=yt[:], scalar1=2.0, scalar2=-1.0,
                op0=Alu.mult, op1=Alu.add,
            )
            t = pool.tile([P, F], f32, tag="ru3")
            nc.vector.tensor_mul(t[:], s[:], yf[:])

            racc = pool.tile([P, KMAX], f32)
            tacc = pool.tile([P, KMAX], f32)
            facc = pool.tile([P, KMAX], f32)
            trash_a = pool.tile([P, F], f32)
            trash_s = pool.tile([P, F], f32, tag="ru4")

            sig_bins = [k for k in range(1, KMAX + 1) if k not in DVE_BINS]

            # a few sigmoid count passes first: they only need zt, so ACT
            # starts ~3us in while the DVE preprocessing chain runs
            def emit_sig(k):
                j = k - 1
                nc.scalar.activation(
                    out=trash_s[:], in_=zt[:], func=Act.Sigmoid,
                    bias=sigb[:, j:j + 1], scale=30.0,
                    accum_out=facc[:, j:j + 1],
                )

            head = sig_bins[:3]
            for k in head:
                emit_sig(k)
            v = pool.tile([P, F], f32)
            nc.scalar.activation(
                out=v[:], in_=t[:], func=Act.Sigmoid,
                bias=negk[:, KMAX:KMAX + 1], scale=1.0,  # bias 0.0
            )
            for k in sig_bins[3:]:
                emit_sig(k)
            x = pool.tile([P, F], f32)
            nc.vector.tensor_add(x[:], v[:], zt[:])
            # exact count anchors on DVE: is_ge at fp16 4x + 2x fold tree
            # instead of the 1x accumulate path (counts <= 2048 stay exact
            # in fp16; the final global fold level is fp32)
            f16 = mybir.dt.float16
            x16 = pool.tile([P, F], f16, tag="ru2")
            nc.vector.tensor_copy(x16[:], x[:])
            dbins = sorted(DVE_BINS)
            nbins = len(dbins)
            cmp16 = pool.tile([P, F], f16, tag="ru1")
            fb1 = pool.tile([P, F // 2], f16, tag="ru3")
            fb2 = pool.tile([P, F // 4], f16)
            fb3 = pool.tile([P, F // 8], f16)
            RW = F // 16  # 256: remnant width per bin
            remn = pool.tile([P, nbins * RW], f16, tag="ru4")
            for bi, k in enumerate(dbins):
                nc.vector.tensor_scalar(
                    out=cmp16[:], in0=x16[:], scalar1=float(k) + 0.5,
                    scalar2=None, op0=Alu.is_ge,
                )
                nc.vector.tensor_add(fb1[:], cmp16[:, :F // 2], cmp16[:, F // 2:])
                nc.vector.tensor_add(fb2[:], fb1[:, :F // 4], fb1[:, F // 4:])
                nc.vector.tensor_add(fb3[:], fb2[:, :F // 8], fb2[:, F // 8:])
                nc.vector.tensor_add(
                    remn[:, bi * RW:(bi + 1) * RW],
                    fb3[:, :F // 16], fb3[:, F // 16:])
            # global fold cascade over all bins' remnants: [P, nbins, w]
            gb = remn
            w = RW
            while w > 1:
                half = w // 2
                src = gb[:].rearrange("p (g d) -> p g d", d=w)
                dt_lvl = f16 if half >= 2 else f32  # last level bound 4096
                # cascade levels reuse slots of tiles dead by this point
                _tg = {128: "ru5", 64: "ru3", 32: "ru1", 16: "ru2"}.get(half, f"gfold{w}")
                dst_t = pool.tile([P, nbins * half], dt_lvl,
                                  name=f"gfold{w}", tag=_tg)
                dst = dst_t[:].rearrange("p (g d) -> p g d", d=half)
                nc.vector.tensor_add(dst, src[:, :, :half], src[:, :, half:])
                gb = dst_t
                w = half
            # gb is [P, nbins] with T_k per partition for dbins order
            nc.vector.tensor_copy(tacc[:, 0:nbins], gb[:])
            # relu value block on ACT (one activation-table switch total)
            for k in range(1, KMAX + 1):
                j = k - 1
                nc.scalar.activation(
                    out=trash_a[:], in_=x[:], func=Act.Relu,
                    bias=negk[:, j:j + 1], scale=1.0,
                    accum_out=racc[:, j:j + 1],
                )

            nc.sync.dma_start(out=out_d[:, 0:KMAX], in_=racc[:])
            nc.sync.dma_start(out=out_d[:, KMAX:2 * KMAX], in_=tacc[:])
            nc.sync.dma_start(out=out_d[:, 2 * KMAX:3 * KMAX], in_=facc[:])

    nc.compile()
    return nc


def _get_program():
    key = ("prog", tuple(sorted(DVE_BINS)))
    if key not in _prog_cache:
        _prog_cache[key] = _build_program()
    return _prog_cache[key]


def _consts():
    negk = np.concatenate(
        [-(np.arange(1, KMAX + 1, dtype=np.float32) + 0.5), np.zeros(1, np.float32)])
    sigb = -30.0 * (np.arange(1, KMAX + 1, dtype=np.float32) + 0.5)
    return (np.broadcast_to(negk, (P, KMAX + 1)).copy(),
            np.broadcast_to(sigb, (P, KMAX)).copy())


def kernel(y_pred: np.ndarray, y: np.ndarray, voronoi: np.ndarray) -> np.ndarray:
    y_pred = np.asarray(y_pred, dtype=np.float32)
    y = np.asarray(y)
    voronoi = np.asarray(voronoi)

    nc = _get_program()
    negk, sigb = _consts()

    in_maps = []
    for c in range(NCORES):
        b = c // CORES_PER_SAMPLE
        q = c % CORES_PER_SAMPLE
        sl = slice(q * CHUNK, (q + 1) * CHUNK)
        zp = y_pred[b].reshape(C, N)
        in_maps.append({
            "z0": np.ascontiguousarray(zp[0, sl]).reshape(P, F),
            "z1": np.ascontiguousarray(zp[1, sl]).reshape(P, F),
            "yb": np.ascontiguousarray(
                y[b, 0].reshape(N)[sl]).astype(np.uint8).reshape(P, F),
            "vor": np.ascontiguousarray(
                voronoi[b].reshape(N)[sl]).astype(np.uint8).reshape(P, F),
            "negk": negk,
            "sigb": sigb,
        })

    res = bass_utils.run_bass_kernel_spmd(
        nc, in_maps, core_ids=list(range(NCORES)), trace=TRACE,
    )
    kernel.last_results = res

    # ---- host-side gather/unshard: combine per-core partials ----
    R = np.zeros((B, KMAX + 2), dtype=np.float64)
    Tm = np.zeros((B, KMAX + 2), dtype=np.float64)
    Fm = np.zeros((B, KMAX + 2), dtype=np.float64)
    for c in range(NCORES):
        b = c // CORES_PER_SAMPLE
        out = np.asarray(res.results[c]["out"], dtype=np.float64)
        R[b, 1:KMAX + 1] += out[:, 0:KMAX].sum(axis=0)
        for bi, kk in enumerate(sorted(DVE_BINS)):
            Tm[b, kk] += out[:, KMAX + bi].sum(axis=0)
        Fm[b, 1:KMAX + 1] += out[:, 2 * KMAX:3 * KMAX].sum(axis=0)

    scores = []
    for b in range(B):
        cnt = np.zeros(KMAX + 2)
        T = np.zeros(KMAX + 2)          # reconstructed T_k, T_65 = 0
        for k in range(KMAX, 0, -1):
            if k in DVE_BINS:
                T[k] = Tm[b, k]
                cnt[k] = T[k] - T[k + 1]
            else:
                cnt[k] = 2.0 * (Fm[b, k] - T[k + 1])
                T[k] = T[k + 1] + cnt[k]
        k = np.arange(1, KMAX + 1)
        M = R[b, k] - R[b, k + 1]
        inter = M - T[k + 1]
        cntk = cnt[k]
        # counts are integers; snap to kill sigmoid-chain noise
        cntk = np.round(cntk)
        dice = (2.0 * inter + EPS) / (2.0 * cntk + EPS)
        present = cntk > 0
        n_present = max(present.sum(), 1)
        scores.append(np.where(present, dice, 0.0).sum() / n_present)

    return np.float32(np.mean(scores))



# revision 10
# speedup vs baseline: 1.6272x; 1.6272x over previous
"""Trainium2 Bass kernel for nn_CCMetrics (connected-component soft-Dice).

Math
----
Reference per sample: probs = softmax(y_pred, ch axis 1) with C=2 channels,
one-hot labels y in {0,1}.  Per-voxel channel sums collapse:
  psum_v = tsum_v = 1          (softmax / one-hot sum to 1 over channels)
  inter_v = probs[true_ch] = sigmoid((2y-1) * (z1 - z0))
So per segment id k (voronoi component, 0..64):
  inter_k = sum of sigmoid values over voxels with id k
  cnt_k   = voxel count with id k
  dice_k  = (2*inter_k + eps) / (2*cnt_k + eps)
  score   = mean over present k in 1..64;  output = mean over batch.

Device algorithm (per core, data-parallel over 4M voxels / 8 cores)
------------------------------------------------------------------
Packed stream per voxel (id g, value v = sigmoid(+-(z1-z0))):
  ztx = 2g + 1                  (odd integers, exact in fp16)
  X   = ztx + v   in fp16       (v lands in the gap [2g+1, 2g+2];
                                 counts stay exact, v quantized ~1/16)
Two per-bin families, k = 1..64:
  R_k = sum relu(X - (2k+1))    value family; M_k = R_k - R_{k+1}
                                 = inter~_k + 2*T_{k+1}
  T_k = #{X >= 2k} = #{g >= k}  count family (exact)
Work is split over THREE engines (the baseline used only ACT+DVE):
  * ACT: self-contained accumulate passes: Relu on X for R_k, and
    steep sigmoid(20*(ztx - 2k)) on ztx for T_k (exact saturation).
  * DVE: preprocessing + cheap 4x fp16 tensor_scalar tiles:
    is_ge masks (counts) and sub+max relu tiles (values).
  * PE (tensor engine, idle in baseline): reduces each DVE tile with
    4 matmuls (moving 1024 cols) against a one-hot [128,64] stationary,
    accumulating every bin into shared PSUM tiles [64, 1024].
Host recovers inter/cnt from R/T in float64 and finishes the dice mean.
"""

import os
import sys

import numpy as np

for _p in ("/opt/trn_rl_repo",):
    if os.path.isdir(_p) and _p not in sys.path:
        sys.path.insert(0, _p)

from concourse import bacc, bass, mybir, tile  # noqa: E402
from concourse import bass_utils  # noqa: E402

NUM_COMP = 64
EPS = 1e-5
B, C, H, W, D = 2, 2, 128, 128, 128
N = H * W * D
NCORES = 8
CORES_PER_SAMPLE = NCORES // B
CHUNK = N // CORES_PER_SAMPLE
P = 128
F = CHUNK // P          # 4096
K = NUM_COMP            # 64 foreground bins
NCH = 8                 # moving chunks per PE reduce
FCH = F // NCH          # 512 (max moving free dim per matmul)

# ---- engine assignment knobs (tunable via env) ----
_ACT_CNT = int(os.environ.get("CC_ACT_CNT", "20"))  # count units on ACT
_ACT_VAL = int(os.environ.get("CC_ACT_VAL", "19"))  # value units on ACT
_ACT_HEAD = int(os.environ.get("CC_ACT_HEAD", "3"))  # ACT cnt units before v
_DVE_HEAD = int(os.environ.get("CC_DVE_HEAD", "11"))  # PE cnt gens before d
_NFOLD = int(os.environ.get("CC_FOLD", "0"))  # PE cnt units w/ DVE pre-fold
_MBUFS = int(os.environ.get("CC_MBUFS", "5"))  # mask buffers per family

ACT_CNT_KS = list(range(1, _ACT_CNT + 1))
PE_CNT_KS = list(range(_ACT_CNT + 1, K + 1))
ACT_VAL_KS = list(range(1, _ACT_VAL + 1))
PE_VAL_KS = list(range(_ACT_VAL + 1, K + 1))

TRACE = False

_prog_cache = {}


def _build_program():
    nc = bacc.Bacc(
        "TRN2",
        target_bir_lowering=False,
        debug=False,
        enable_asserts=False,
        num_devices=NCORES,
    )
    f32 = mybir.dt.float32
    f16 = mybir.dt.float16

    z0_d = nc.dram_tensor("z0", [P, F], f32, kind="ExternalInput").ap()
    z1_d = nc.dram_tensor("z1", [P, F], f32, kind="ExternalInput").ap()
    yf_d = nc.dram_tensor("yf", [P, F], f16, kind="ExternalInput").ap()
    ztx_d = nc.dram_tensor("ztx", [P, F], f16, kind="ExternalInput").ap()
    oneh_d = nc.dram_tensor("oneh", [P, K * K], f16, kind="ExternalInput").ap()
    # bias constants: cols 0..63 relu bias -(2k+1); cols 64..127 sigmoid
    # bias -40k; col 128: 0.0
    bias_d = nc.dram_tensor("bias", [P, 2 * K + 1], f32, kind="ExternalInput").ap()
    racc_d = nc.dram_tensor("racc", [P, K], f32, kind="ExternalOutput").ap()
    facc_d = nc.dram_tensor("facc", [P, K], f32, kind="ExternalOutput").ap()
    pval_d = nc.dram_tensor("pval", [K, FCH], f32, kind="ExternalOutput").ap()
    pcnt_d = nc.dram_tensor("pcnt", [K, FCH], f32, kind="ExternalOutput").ap()

    Alu = mybir.AluOpType
    Act = mybir.ActivationFunctionType

    with tile.TileContext(nc) as tc:
        with tc.tile_pool(name="main", bufs=1) as pool, \
             tc.tile_pool(name="mask", bufs=1) as mpool, \
             tc.tile_pool(name="psum", bufs=1, space="PSUM") as ppool:
            ztx = pool.tile([P, F], f16)
            yf = pool.tile([P, F], f16)
            oneh = pool.tile([P, K * K], f16)
            bias = pool.tile([P, 2 * K + 1], f32)
            z0 = pool.tile([P, F], f32)
            z1 = pool.tile([P, F], f32)
            # small tensors first so early ACT/DVE work is not gated on the
            # 4MB z DMAs
            nc.sync.dma_start(out=ztx[:], in_=ztx_d[:])
            nc.sync.dma_start(out=bias[:], in_=bias_d[:])
            nc.sync.dma_start(out=yf[:], in_=yf_d[:])
            nc.sync.dma_start(out=oneh[:], in_=oneh_d[:])
            nc.sync.dma_start(out=z0[:], in_=z0_d[:])
            nc.sync.dma_start(out=z1[:], in_=z1_d[:])

            pv = ppool.tile([K, FCH], f32)
            pc = ppool.tile([K, FCH], f32)
            racc = pool.tile([P, K], f32)
            facc = pool.tile([P, K], f32)
            trash_a = pool.tile([P, F], f16)

            # --- PE accumulation bookkeeping: start=True only on the very
            # first matmul touching a region (resets PSUM), stop=True on the
            # very last (sim requirement). ---
            n_pe_mm = {id(pv): 0, id(pc): 0}
            tot_pe_mm = {
                id(pv): len(PE_VAL_KS) * NCH,
                id(pc): (len(PE_CNT_KS) - _NFOLD) * NCH + _NFOLD * (NCH // 2),
            }

            def pe_reduce(m, region, row, nch):
                fch = F // nch
                lhs = oneh[:, K * row:K * row + K]
                for c in range(nch):
                    i = n_pe_mm[id(region)]
                    nc.tensor.matmul(
                        region[:],
                        lhs,
                        m[:, c * fch:(c + 1) * fch],
                        start=(i == 0),
                        stop=(i == tot_pe_mm[id(region)] - 1),
                        skip_group_check=True,
                    )
                    n_pe_mm[id(region)] += 1

            def act_cnt(k):
                nc.scalar.activation(
                    out=trash_a[:], in_=ztx[:], func=Act.Sigmoid,
                    bias=bias[:, K + k - 1:K + k], scale=20.0,
                    accum_out=facc[:, k - 1:k],
                )

            def act_val(k):
                nc.scalar.activation(
                    out=trash_a[:], in_=x[:], func=Act.Relu,
                    bias=bias[:, k - 1:k], scale=1.0,
                    accum_out=racc[:, k - 1:k],
                )

            def gen_cnt(k, fold=False):
                m = mpool.tile([P, F], f16, tag="cmask", bufs=_MBUFS,
                               name=f"cm{k}")
                nc.vector.tensor_scalar(
                    out=m[:], in0=ztx[:], scalar1=float(2 * k), scalar2=None,
                    op0=Alu.is_ge,
                )
                if not fold:
                    pe_reduce(m, pc, k - 1, NCH)
                    return
                mf = mpool.tile([P, F // 2], f16, tag="fmask", bufs=_MBUFS,
                                name=f"cf{k}")
                nc.vector.tensor_add(mf[:], m[:, :F // 2], m[:, F // 2:])
                pe_reduce(mf, pc, k - 1, NCH // 2)

            def gen_val(k):
                m = mpool.tile([P, F], f16, tag="vmask", bufs=_MBUFS,
                               name=f"vm{k}")
                nc.vector.tensor_scalar(
                    out=m[:], in0=x[:], scalar1=float(2 * k + 1), scalar2=0.0,
                    op0=Alu.subtract, op1=Alu.max,
                )
                pe_reduce(m, pv, k - 1, NCH)

            # ---- emission order = per-engine FIFO order ----
            # ACT head: sigmoid count units (need only ztx)
            for k in ACT_CNT_KS[:_ACT_HEAD]:
                act_cnt(k)
            # DVE head: count-mask gens + PE reduces (need only ztx) to keep
            # DVE/PE busy while the z DMAs stream in
            fold_ks = set(PE_CNT_KS[-_NFOLD:]) if _NFOLD else set()
            for k in PE_CNT_KS[:_DVE_HEAD]:
                gen_cnt(k, fold=k in fold_ks)
            # preprocessing: d = z1 - z0, t = d * (2y-1)
            d16 = pool.tile([P, F], f16)
            nc.vector.tensor_sub(d16[:], z1[:], z0[:])
            t16 = pool.tile([P, F], f16)
            nc.vector.tensor_mul(t16[:], d16[:], yf[:])
            # v = sigmoid(t) on ACT (after its head count units)
            v16 = pool.tile([P, F], f16)
            nc.scalar.activation(
                out=v16[:], in_=t16[:], func=Act.Sigmoid,
                bias=bias[:, 2 * K:2 * K + 1], scale=1.0,
            )
            # a couple more DVE count gens while ACT computes v
            for k in PE_CNT_KS[_DVE_HEAD:_DVE_HEAD + 2]:
                gen_cnt(k, fold=k in fold_ks)
            # X = ztx + v
            x = pool.tile([P, F], f16)
            nc.vector.tensor_add(x[:], ztx[:], v16[:])
            # remaining ACT units: counts first (no X dep), then values
            for k in ACT_CNT_KS[_ACT_HEAD:]:
                act_cnt(k)
            for k in ACT_VAL_KS:
                act_val(k)
            # remaining DVE->PE units: interleave values with leftover counts
            rest_cnt = PE_CNT_KS[_DVE_HEAD + 2:]
            rest_val = list(PE_VAL_KS)
            seq = []
            nc_, nv_ = len(rest_cnt), len(rest_val)
            ci = vi = 0
            for i in range(nc_ + nv_):
                # spread counts evenly among values
                if ci < nc_ and (vi >= nv_ or ci * nv_ <= vi * nc_):
                    seq.append(("c", rest_cnt[ci])); ci += 1
                else:
                    seq.append(("v", rest_val[vi])); vi += 1
            for kind, k in seq:
                if kind == "c":
                    gen_cnt(k, fold=k in fold_ks)
                else:
                    gen_val(k)

            # ---- drain: PSUM -> SBUF -> DRAM, ACT accumulators -> DRAM ----
            pv_sb = pool.tile([K, FCH], f32)
            pc_sb = pool.tile([K, FCH], f32)
            nc.vector.tensor_copy(pv_sb[:], pv[:])
            nc.vector.tensor_copy(pc_sb[:], pc[:])
            nc.sync.dma_start(out=pval_d[:], in_=pv_sb[:])
            nc.sync.dma_start(out=pcnt_d[:], in_=pc_sb[:])
            nc.sync.dma_start(out=racc_d[:], in_=racc[:])
            nc.sync.dma_start(out=facc_d[:], in_=facc[:])

    nc.compile()
    return nc


def _get_program():
    key = ("prog", _ACT_CNT, _ACT_VAL, _ACT_HEAD, _DVE_HEAD, _NFOLD, _MBUFS)
    if key not in _prog_cache:
        _prog_cache[key] = _build_program()
    return _prog_cache[key]


def _onehot_const():
    oneh = np.zeros((P, K * K), dtype=np.float16)
    for j in range(K):
        oneh[:, K * j + j] = 1.0
    return oneh


def _bias_const():
    ks = np.arange(1, K + 1, dtype=np.float32)
    row = np.concatenate([-(2 * ks + 1), -40.0 * ks, np.zeros(1, np.float32)])
    return np.broadcast_to(row, (P, 2 * K + 1)).copy()


def kernel(y_pred: np.ndarray, y: np.ndarray, voronoi: np.ndarray) -> np.ndarray:
    y_pred = np.asarray(y_pred, dtype=np.float32)
    y = np.asarray(y)
    voronoi = np.asarray(voronoi)

    nc = _get_program()
    oneh = _onehot_const()
    biasc = _bias_const()

    in_maps = []
    for c in range(NCORES):
        b = c // CORES_PER_SAMPLE
        q = c % CORES_PER_SAMPLE
        sl = slice(q * CHUNK, (q + 1) * CHUNK)
        zp = y_pred[b].reshape(C, N)
        yv = y[b, 0].reshape(N)[sl].astype(np.int32)
        gv = voronoi[b].reshape(N)[sl].astype(np.int32)
        in_maps.append({
            "z0": np.ascontiguousarray(zp[0, sl]).reshape(P, F),
            "z1": np.ascontiguousarray(zp[1, sl]).reshape(P, F),
            "yf": (2 * yv - 1).astype(np.float16).reshape(P, F),
            "ztx": (2 * gv + 1).astype(np.float16).reshape(P, F),
            "oneh": oneh,
            "bias": biasc,
        })

    res = bass_utils.run_bass_kernel_spmd(
        nc, in_maps, core_ids=list(range(NCORES)), trace=TRACE,
    )
    kernel.last_results = res

    # ---- host-side gather/unshard: combine per-core partials (float64) ----
    R = np.zeros((B, K + 2), dtype=np.float64)
    T = np.zeros((B, K + 2), dtype=np.float64)
    for c in range(NCORES):
        b = c // CORES_PER_SAMPLE
        out = res.results[c]
        racc = np.asarray(out["racc"], dtype=np.float64)
        facc = np.asarray(out["facc"], dtype=np.float64)
        pval = np.asarray(out["pval"], dtype=np.float64)
        pcnt = np.asarray(out["pcnt"], dtype=np.float64)
        for k in ACT_VAL_KS:
            R[b, k] += racc[:, k - 1].sum()
        for k in PE_VAL_KS:
            R[b, k] += pval[k - 1, :].sum()
        for k in ACT_CNT_KS:
            T[b, k] += facc[:, k - 1].sum()
        for k in PE_CNT_KS:
            T[b, k] += pcnt[k - 1, :].sum()

    scores = []
    ks = np.arange(1, K + 1)
    for b in range(B):
        M = R[b, ks] - R[b, ks + 1]          # R[65] = 0
        inter = M - 2.0 * T[b, ks + 1]       # T[65] = 0
        cnt = np.round(T[b, ks] - T[b, ks + 1])
        dice = (2.0 * inter + EPS) / (2.0 * cnt + EPS)
        present = cnt > 0
        n_present = max(present.sum(), 1)
        scores.append(np.where(present, dice, 0.0).sum() / n_present)

    return np.float32(np.mean(scores))
